# revision 17
# baseline (speedup 1.0000x reference)
"""Multi-head attention (B=2, L=2048, D=2048, 16 heads of 128) on 8 NeuronCores.

v4: engine-balance + boundary-latency pass over v3.
- All matmul operands bf16 (FWL on every stationary, halved x DMA).
- Softmax denominators: exp tiles pair-summed and accumulated on the POOL
  engine (otherwise idle), one ones-matmul per head-chunk reduces partitions.
- PV psum is normalized directly (no o_raw SBUF copy); reciprocal output is
  bitcast to f32r for the broadcast matmul (drops one DVE hop from the
  head-boundary chain).
- oproj fillers split 6/2 across the two heads' k-loops in phase 3 so the PE
  has independent work over every head-boundary norm chain.
- yst copies all on DVE (scalar engine is exp-bound in phase 3).
- Phase-1 DMA: wq streamed as per-head halves ahead of everything, x chunk 2
  on the sync queue; ~96 tiny warmup matmuls hold HAM at K=8/8 until real
  matmuls start.
"""

import numpy as np
from contextlib import ExitStack

import concourse.bacc as bacc
import concourse.tile as tile
from concourse import mybir
from concourse.bass_utils import run_bass_kernel_spmd

F32 = mybir.dt.float32
F32R = mybir.dt.float32r
BF16 = mybir.dt.bfloat16
Exp = mybir.ActivationFunctionType.Exp

B, L, D, H, HD = 2, 2048, 2048, 16, 128
NCORES = 8
HPC = H // NCORES  # 2 heads per core
EC = HPC * HD  # 256 per-core projection width
ND = D // 128  # 16 contraction tiles
QC = 512  # q chunk
NQC = L // QC  # 4 chunks per batch
NKT = L // 128  # 16 k tiles per batch
SCALE = float(HD) ** -0.5

_nc_cache = None


def _build():
    nc = bacc.Bacc()
    xt = nc.dram_tensor("xt", [D, B * L], BF16, kind="ExternalInput")
    wqt = nc.dram_tensor("wqt", [D, EC], BF16, kind="ExternalInput")
    wkt = nc.dram_tensor("wkt", [D, EC], BF16, kind="ExternalInput")
    wvt = nc.dram_tensor("wvt", [D, EC], BF16, kind="ExternalInput")
    wot = nc.dram_tensor("wot", [EC, D], BF16, kind="ExternalInput")
    cost = nc.dram_tensor("cost", [HD, L], BF16, kind="ExternalInput")
    sst = nc.dram_tensor("sst", [HD, L], BF16, kind="ExternalInput")
    yt = nc.dram_tensor("yt", [D, B * L], BF16, kind="ExternalOutput")

    with tile.TileContext(nc) as tc, ExitStack() as ctx:
        persist = ctx.enter_context(tc.tile_pool(name="persist", bufs=1))
        qk = ctx.enter_context(tc.tile_pool(name="qk", bufs=1))
        xr = ctx.enter_context(tc.tile_pool(name="xr", bufs=36))
        st = ctx.enter_context(tc.tile_pool(name="st", bufs=2))
        exp_pool = ctx.enter_context(tc.tile_pool(name="exp_pool", bufs=2))
        y_pool = ctx.enter_context(tc.tile_pool(name="y_pool", bufs=8))
        np_pool = ctx.enter_context(tc.tile_pool(name="np_pool", bufs=2))
        psp = ctx.enter_context(tc.tile_pool(name="psp", bufs=1, space="PSUM"))

        # psum tags: C = [128,1024] x3 (scores pairs, proj passes, v-pass,
        # oproj e-pairs), O = [128,512] x1 (PV accum, freed by norm mul),
        # U = [128,512] x1 (denominator row + rcp broadcast + warmup)
        def ps(tag, shape=(128, QC), bufs=None):
            return psp.tile(list(shape), F32, tag=tag, name=f"ps{tag}", bufs=bufs)

        # constants
        onescol = persist.tile([128, 1], F32)
        nc.vector.memset(onescol[:], 1.0)
        ones_k = persist.tile([128, 1], BF16)
        nc.vector.tensor_copy(ones_k[:], onescol[:])
        ones_r = persist.tile([128, 1], F32R)
        nc.vector.tensor_copy(ones_r[:], onescol[:])
        onesrow = persist.tile([1, 128], F32)
        nc.vector.memset(onesrow[:], 1.0)
        ones1 = persist.tile([1, 128], F32R)
        nc.vector.tensor_copy(ones1[:], onesrow[:])

        # PE warmup: tiny matmuls keep the PE busy from engine start so HAM
        # reaches K=8/8 before the first real matmul; sized to drain by the
        # time the first x/wq tiles have landed.
        warm = persist.tile([128, 64], BF16)
        nc.vector.memset(warm[:], 1.0)
        ps_w = ps("U")
        for _ in range(300):
            nc.tensor.matmul(ps_w[0:1, 0:64], ones_k[:], warm[:], start=True, stop=True)

        # sync queue: wq interleaved with chunk-0 even x tiles, then wk, wv,
        # wo. gpsimd queue: chunk-0 odd x tiles, cos/sin, the other batch-0
        # x chunks. Batch-1 x chunks go via the vector queue; y writes own
        # the sync queue in phases 2-3.
        w_sb = {}
        x_tiles = [None] * ND
        for t in range(ND):
            w_t = persist.tile([128, EC], BF16, tag=f"wq{t}", name=f"wq{t}")
            nc.sync.dma_start(w_t[:], wqt[t * 128 : (t + 1) * 128, :])
            w_sb["q", t] = w_t
        for t in range(ND):
            x_t = xr.tile([128, QC], BF16, tag="x")
            nc.gpsimd.dma_start(x_t[:], xt[t * 128 : (t + 1) * 128, 0:QC])
            x_tiles[t] = x_t
        for t in range(ND):
            w_t = persist.tile([128, EC], BF16, tag=f"wk{t}", name=f"wk{t}")
            nc.sync.dma_start(w_t[:], wkt[t * 128 : (t + 1) * 128, :])
            w_sb["k", t] = w_t
        cos_sb = persist.tile([128, L], BF16)
        sw_sb = persist.tile([128, L], BF16)
        nc.sync.dma_start(cos_sb[:], cost[:])
        nc.sync.dma_start(sw_sb[:], sst[:])
        for t in range(ND):
            w_t = persist.tile([128, EC], BF16, tag=f"wv{t}", name=f"wv{t}")
            nc.sync.dma_start(w_t[:], wvt[t * 128 : (t + 1) * 128, :])
            w_sb["v", t] = w_t
        wo_sb = persist.tile([128, HPC * D], BF16)
        for h in range(HPC):
            nc.sync.dma_start(
                wo_sb[:, h * D : (h + 1) * D], wot[h * 128 : (h + 1) * 128, :]
            )

        # per-(batch, head, chunk) q/k bf16; per-(batch, chunk, lblock) v bf16
        qT = [[[None] * NQC for _ in range(HPC)] for _ in range(B)]
        kT = [[[None] * NQC for _ in range(HPC)] for _ in range(B)]
        v_sb = [[[None] * 4 for _ in range(NQC)] for _ in range(B)]

        def emit_x_dma(b, c, queue=None):
            for t in range(ND):
                q = queue if queue is not None else (nc.sync if t % 2 == 0 else nc.scalar)
                x_t = xr.tile([128, QC], BF16, tag="x")
                q.dma_start(
                    x_t[:],
                    xt[t * 128 : (t + 1) * 128, b * L + c * QC : b * L + (c + 1) * QC],
                )
                x_tiles[t] = x_t

        def qk_pass_mm(b, c, kind):
            raws = []
            for h in range(HPC):
                psq2 = ps("C", (128, 2 * QC), bufs=3)
                psq = psq2[:, 0:QC]
                for t in range(ND):
                    nc.tensor.matmul(
                        psq,
                        w_sb[kind, t][:, h * HD : (h + 1) * HD],
                        x_tiles[t][:],
                        start=(t == 0),
                        stop=(t == ND - 1),
                    )
                raw = st.tile([128, QC], F32, tag="raw", bufs=3)
                nc.scalar.copy(raw[:], psq)  # releases psum
                raws.append(raw)
            return raws

        def qk_rope(b, c, kind, raws):
            lsl = slice(c * QC, (c + 1) * QC)
            dst = qT if kind == "q" else kT
            for h in range(HPC):
                raw = raws[h]
                pA = st.tile([128, QC], F32, tag="pA", bufs=1)
                nc.vector.tensor_mul(pA[:], raw[:], cos_sb[:, lsl])
                tmp = st.tile([128, QC], F32, tag="tmp", bufs=2)
                nc.vector.tensor_mul(tmp[0:64, :], raw[64:128, :], sw_sb[64:128, lsl])
                nc.vector.tensor_mul(tmp[64:128, :], raw[0:64, :], sw_sb[0:64, lsl])
                dt_ = qk.tile([128, QC], BF16, tag=f"{kind}T{b}{h}_{c}")
                nc.vector.tensor_add(dt_[:], pA[:], tmp[:])
                dst[b][h][c] = dt_

        def qk_pass(b, c, kind):
            qk_rope(b, c, kind, qk_pass_mm(b, c, kind))

        def v_pass(b, c):
            psv = ps("C", (128, 2 * QC), bufs=3)
            for lb in range(4):
                for t in range(ND):
                    nc.tensor.matmul(
                        psv[:, lb * EC : (lb + 1) * EC],
                        x_tiles[t][:, lb * 128 : (lb + 1) * 128],
                        w_sb["v", t][:],
                        start=(t == 0),
                        stop=(t == ND - 1),
                    )
            for lb in range(4):
                vt = qk.tile([128, EC], BF16, tag=f"v{b}{c}_{lb}")
                nc.vector.tensor_copy(vt[:], psv[:, lb * EC : (lb + 1) * EC])
                v_sb[b][c][lb] = vt

        def attn_head(b, c, h, filler=None):
            """k-loop for one head; returns (o_raw bf16, rcp f32r [1,QC]).

            Denominators: each exp pair is summed to one [128,QC] bf16 tile on
            the Pool engine (slow but off the critical path), then folded into
            PSUM by cheap ones-matmuls on the PE, lagged two pairs so a slow
            Pool op never stalls the PE."""
            ps_o = ps("O")
            ps_u = [None]  # allocated lazily at the first denominator matmul
            prs = []
            qds = []

            def emit_pv(ex, p):
                for j in range(2):
                    t = 2 * p + j
                    nc.tensor.matmul(
                        ps_o[:],
                        v_sb[b][t // 4][t % 4][:, h * HD : (h + 1) * HD],
                        ex[:, j * QC : (j + 1) * QC],
                        start=(t == 0),
                        stop=(t == NKT - 1),
                    )

            def emit_dn(qi):
                if ps_u[0] is None:
                    ps_u[0] = ps("U")
                nc.tensor.matmul(
                    ps_u[0][0:1, :],
                    ones_k[:],
                    qds[qi][:],
                    start=(qi == 0),
                    stop=(qi == NKT // 4 - 1),
                )

            prev = None
            for p in range(NKT // 2):
                if filler and p >= 1:
                    filler.pop(0)()
                ps_s = ps("C", (128, 2 * QC), bufs=3)
                for j in range(2):
                    t = 2 * p + j
                    nc.tensor.matmul(
                        ps_s[:, j * QC : (j + 1) * QC],
                        kT[b][h][t // 4][:, (t % 4) * 128 : (t % 4 + 1) * 128],
                        qT[b][h][c][:],
                        start=True,
                        stop=True,
                    )
                ex = exp_pool.tile([128, 2 * QC], BF16, tag="ex")
                nc.scalar.activation(ex[:], ps_s[:], Exp, scale=SCALE)
                pr = st.tile([128, QC], BF16, tag="dps", bufs=4)
                nc.gpsimd.tensor_add(pr[:], ex[:, 0:QC], ex[:, QC : 2 * QC])
                prs.append(pr)
                if p % 2 == 1:
                    qd = st.tile([128, QC], BF16, tag="dqs", bufs=3)
                    nc.vector.tensor_add(qd[:], prs[p - 1][:], prs[p][:])
                    qds.append(qd)
                if prev is not None:
                    emit_pv(prev, p - 1)
                if p >= 4 and p % 2 == 0:
                    emit_dn(p // 2 - 2)
                prev = ex
            emit_pv(prev, NKT // 2 - 1)
            emit_dn(NKT // 4 - 2)
            emit_dn(NKT // 4 - 1)
            while filler:
                filler.pop(0)()
            o_raw = st.tile([128, QC], BF16, tag="o_raw")
            nc.vector.tensor_copy(o_raw[:], ps_o[:])  # frees O bank
            rcp32 = st.tile([1, QC], F32, tag="rcp32", bufs=2)
            nc.vector.reciprocal_approx_fast(rcp32[:], ps_u[0][0:1, :])  # frees U
            rcp = st.tile([1, QC], F32R, tag="rcpr", bufs=2)
            nc.vector.tensor_copy(rcp[:], rcp32[:])
            return o_raw, rcp

        def norm_step(h, o_raw, rcp, onorm):
            def one():
                ps_b = ps("U")
                nc.tensor.matmul(ps_b[:], ones1[:], rcp[:], start=True, stop=True)
                nc.vector.tensor_mul(onorm[:], o_raw[:], ps_b[:])  # frees U
            return one

        def norm_head(h, o_raw, rcp):
            onorm = np_pool.tile([128, QC], BF16, tag=f"norm{h}", name=f"onorm{h}")
            norm_step(h, o_raw, rcp, onorm)()
            return onorm

        def emit_yst(b, c, e, src_ap, queue=None):
            yst = y_pool.tile([128, QC], BF16, tag="yst")
            nc.vector.tensor_copy(yst[:], src_ap)
            (queue if queue is not None else nc.sync).dma_start(
                yt[e * 128 : (e + 1) * 128, b * L + c * QC : b * L + (c + 1) * QC],
                yst[:],
            )

        def oproj_steps(b, c, norm_tiles):
            steps = []
            for p in range(ND // 2):
                def one(p=p):
                    ps_y2 = ps("C", (128, 2 * QC), bufs=3)
                    for e in (2 * p, 2 * p + 1):
                        off0 = e * 128
                        for h in range(HPC):
                            nc.tensor.matmul(
                                ps_y2[:, (e % 2) * QC : (e % 2 + 1) * QC],
                                wo_sb[:, h * D + off0 : h * D + off0 + 128],
                                norm_tiles[h][:],
                                start=(h == 0),
                                stop=(h == HPC - 1),
                            )
                    for e in (2 * p, 2 * p + 1):
                        emit_yst(b, c, e, ps_y2[:, (e % 2) * QC : (e % 2 + 1) * QC])
                steps.append(one)
            return steps

        def oproj(b, c, norm_tiles, prefill_h0=False):
            # e-pairs share one C tile; optionally emit all h0 (start)
            # matmuls of the first pairs before h1 is ready.
            pairs = [ps("C", (128, 2 * QC), bufs=3) for _ in range(2)]
            emitted = {}

            def mm(p, e, h, ps_y2):
                off = h * D + e * 128
                nc.tensor.matmul(
                    ps_y2[:, (e % 2) * QC : (e % 2 + 1) * QC],
                    wo_sb[:, off : off + 128],
                    norm_tiles[h][:],
                    start=(h == 0),
                    stop=(h == HPC - 1),
                )

            if prefill_h0:
                for p in range(2):
                    for e in (2 * p, 2 * p + 1):
                        mm(p, e, 0, pairs[p])
                        emitted[e] = True
            for p in range(ND // 2):
                ps_y2 = pairs[p] if p < 2 else ps("C", (128, 2 * QC), bufs=3)
                for e in (2 * p, 2 * p + 1):
                    if e not in emitted:
                        mm(p, e, 0, ps_y2)
                    mm(p, e, 1, ps_y2)
                for e in (2 * p, 2 * p + 1):
                    emit_yst(
                        b, c, e,
                        ps_y2[:, (e % 2) * QC : (e % 2 + 1) * QC],
                        queue=nc.sync if e % 2 == 0 else nc.gpsimd,
                    )

        # ---- schedule ----
        # phase 1: batch-0 projections
        for c in range(NQC):
            if c > 0:
                emit_x_dma(0, c, queue=nc.gpsimd)
            qk_pass(0, c, "q")
            qk_pass(0, c, "k")
            v_pass(0, c)

        # phases 2+3: previous chunk's oproj is interleaved into the next
        # chunk's k-loops, one e-pair per score-pair step. Norm broadcasts are
        # emitted only after other PE work (or deferred into the next k-loop
        # as a filler step) so their reciprocal-wait never stalls the PE.
        pend = None  # oproj steps awaiting emission
        for c in range(NQC):
            emit_x_dma(1, c)  # split sync/scalar; 36-buf ring so starts never block
            o0, r0 = attn_head(0, c, 0, filler=pend)
            qk_pass(1, c, "q")
            n0 = norm_head(0, o0, r0)
            o1, r1 = attn_head(0, c, 1)
            kraws = qk_pass_mm(1, c, "k")
            n1 = norm_head(1, o1, r1)
            pend = oproj_steps(0, c, [n0, n1])
            qk_rope(1, c, "k", kraws)
            v_pass(1, c)

        for c in range(NQC):
            if pend:
                f0 = pend[:6]
                if len(pend) == 9:  # norm1-prev at index 0; delay to slot 3
                    f0 = [pend[1], pend[2], pend[0]] + pend[3:6]
                rest = pend[6:]
            else:
                f0, rest = None, []
            o0, r0 = attn_head(1, c, 0, filler=f0)
            n0 = np_pool.tile([128, QC], BF16, tag="norm0", name=f"pn0_{c}")
            f1 = rest[:2] + [norm_step(0, o0, r0, n0)] + rest[2:]
            o1, r1 = attn_head(1, c, 1, filler=f1)
            n1 = np_pool.tile([128, QC], BF16, tag="norm1", name=f"pn1_{c}")
            if c < NQC - 1:
                pend = [norm_step(1, o1, r1, n1)] + oproj_steps(1, c, [n0, n1])
            else:
                ps_t = ps("C", (128, 2 * QC), bufs=3)
                for _ in range(24):  # keep HAM warm across the tail norm chain
                    nc.tensor.matmul(
                        ps_t[0:1, 0:64], ones_k[:], warm[:], start=True, stop=True
                    )
                norm_step(1, o1, r1, n1)()
                oproj(1, c, [n0, n1], prefill_h0=True)
    nc.finalize()
    return nc


def _get_nc():
    global _nc_cache
    if _nc_cache is None:
        _nc_cache = _build()
    return _nc_cache


def _prepare_in_maps(inputs):
    x = np.asarray(inputs["x"], np.float32)
    rope = np.asarray(inputs["rope_emb"], np.float32)
    wq = np.asarray(inputs["wq"], np.float32)
    wk = np.asarray(inputs["wk"], np.float32)
    wv = np.asarray(inputs["wv"], np.float32)
    wo = np.asarray(inputs["wo"], np.float32)

    import ml_dtypes

    BF = ml_dtypes.bfloat16
    xt = np.ascontiguousarray(x.reshape(B * L, D).T.astype(BF))
    cosT = np.ascontiguousarray(np.cos(rope).T)  # [HD, L]
    sinT = np.sin(rope).T  # [HD, L]
    sinsT = np.concatenate([-sinT[: HD // 2], sinT[HD // 2 :]], 0)
    # partition-rotated so rope reads stay partition-aligned on device
    ssw = np.ascontiguousarray(
        np.concatenate([sinsT[HD // 2 :], sinsT[: HD // 2]], 0)
    )
    cosT = cosT.astype(BF)
    ssw = ssw.astype(BF)

    in_maps = []
    for c in range(NCORES):
        rows = slice(c * EC, (c + 1) * EC)
        in_maps.append(
            {
                "xt": xt,
                "cost": cosT,
                "sst": ssw,
                "wqt": np.ascontiguousarray(wq[rows].T.astype(BF)),
                "wkt": np.ascontiguousarray(wk[rows].T.astype(BF)),
                "wvt": np.ascontiguousarray(wv[rows].T.astype(BF)),
                "wot": np.ascontiguousarray(wo[:, rows].T.astype(BF)),
            }
        )
    return in_maps


def kernel(**inputs):
    bo = np.asarray(inputs["bo"], np.float32)
    in_maps = _prepare_in_maps(inputs)
    nc = _get_nc()
    res = run_bass_kernel_spmd(nc, in_maps, core_ids=list(range(NCORES)))
    y_t = np.asarray(res.results[0]["yt"], dtype=np.float32)
    for c in range(1, NCORES):
        y_t += np.asarray(res.results[c]["yt"], dtype=np.float32)
    y = y_t.T.reshape(B, L, D) + bo[None, None, :]
    return y.astype(np.float32)


# revision 19
# speedup vs baseline: 1.1763x; 1.1763x over previous
"""Multi-head attention (B=2, L=2048, D=2048, 16 heads of 128) on 8 NeuronCores.

v4: engine-balance + boundary-latency pass over v3.
- All matmul operands bf16 (FWL on every stationary, halved x DMA).
- Softmax denominators: exp tiles pair-summed and accumulated on the POOL
  engine (otherwise idle), one ones-matmul per head-chunk reduces partitions.
- PV psum is normalized directly (no o_raw SBUF copy); reciprocal output is
  bitcast to f32r for the broadcast matmul (drops one DVE hop from the
  head-boundary chain).
- oproj fillers split 6/2 across the two heads' k-loops in phase 3 so the PE
  has independent work over every head-boundary norm chain.
- yst copies all on DVE (scalar engine is exp-bound in phase 3).
- Phase-1 DMA: wq streamed as per-head halves ahead of everything, x chunk 2
  on the sync queue; ~96 tiny warmup matmuls hold HAM at K=8/8 until real
  matmuls start.
"""

import numpy as np
from contextlib import ExitStack

import concourse.bacc as bacc
import concourse.tile as tile
from concourse import mybir
from concourse.bass_utils import run_bass_kernel_spmd

F32 = mybir.dt.float32
F32R = mybir.dt.float32r
BF16 = mybir.dt.bfloat16
Exp = mybir.ActivationFunctionType.Exp

B, L, D, H, HD = 2, 2048, 2048, 16, 128
NCORES = 8
HPC = H // NCORES  # 2 heads per core
EC = HPC * HD  # 256 per-core projection width
ND = D // 128  # 16 contraction tiles
QC = 512  # q chunk
NQC = L // QC  # 4 chunks per batch
NKT = L // 128  # 16 k tiles per batch
SCALE = float(HD) ** -0.5

_nc_cache = None


def _build():
    nc = bacc.Bacc()
    xt = nc.dram_tensor("xt", [D, B * L], BF16, kind="ExternalInput")
    wqt = nc.dram_tensor("wqt", [D, EC], BF16, kind="ExternalInput")
    wkt = nc.dram_tensor("wkt", [D, EC], BF16, kind="ExternalInput")
    wvt = nc.dram_tensor("wvt", [D, EC], BF16, kind="ExternalInput")
    wot = nc.dram_tensor("wot", [EC, D], BF16, kind="ExternalInput")
    cost = nc.dram_tensor("cost", [HD, L], BF16, kind="ExternalInput")
    sst = nc.dram_tensor("sst", [HD, L], BF16, kind="ExternalInput")
    yt = nc.dram_tensor("yt", [D, B * L], BF16, kind="ExternalOutput")

    with tile.TileContext(nc) as tc, ExitStack() as ctx:
        persist = ctx.enter_context(tc.tile_pool(name="persist", bufs=1))
        qk = ctx.enter_context(tc.tile_pool(name="qk", bufs=1))
        xr = ctx.enter_context(tc.tile_pool(name="xr", bufs=36))
        st = ctx.enter_context(tc.tile_pool(name="st", bufs=2))
        exp_pool = ctx.enter_context(tc.tile_pool(name="exp_pool", bufs=4))
        y_pool = ctx.enter_context(tc.tile_pool(name="y_pool", bufs=8))
        np_pool = ctx.enter_context(tc.tile_pool(name="np_pool", bufs=2))
        psp = ctx.enter_context(tc.tile_pool(name="psp", bufs=1, space="PSUM"))

        # psum tags: C = [128,1024] x3 (scores pairs, proj passes, v-pass,
        # oproj e-pairs), O = [128,512] x1 (PV accum, freed by norm mul),
        # U = [128,512] x1 (denominator row + rcp broadcast + warmup)
        def ps(tag, shape=(128, QC), bufs=None):
            return psp.tile(list(shape), F32, tag=tag, name=f"ps{tag}", bufs=bufs)

        # constants
        onescol = persist.tile([128, 1], F32)
        nc.vector.memset(onescol[:], 1.0)
        ones_k = persist.tile([128, 1], BF16)
        nc.vector.tensor_copy(ones_k[:], onescol[:])
        ones_r = persist.tile([128, 1], F32R)
        nc.vector.tensor_copy(ones_r[:], onescol[:])
        onesrow = persist.tile([1, 128], F32)
        nc.vector.memset(onesrow[:], 1.0)
        ones1 = persist.tile([1, 128], F32R)
        nc.vector.tensor_copy(ones1[:], onesrow[:])

        # PE warmup: tiny matmuls keep the PE busy from engine start so HAM
        # reaches K=8/8 before the first real matmul; sized to drain by the
        # time the first x/wq tiles have landed.
        warm = persist.tile([128, 64], BF16)
        nc.vector.memset(warm[:], 1.0)
        ps_w = ps("U")
        for _ in range(300):
            nc.tensor.matmul(ps_w[0:1, 0:64], ones_k[:], warm[:], start=True, stop=True)

        # sync queue: wq interleaved with chunk-0 even x tiles, then wk, wv,
        # wo. gpsimd queue: chunk-0 odd x tiles, cos/sin, the other batch-0
        # x chunks. Batch-1 x chunks go via the vector queue; y writes own
        # the sync queue in phases 2-3.
        w_sb = {}
        x_tiles = [None] * ND
        for t in range(ND):
            w_t = persist.tile([128, EC], BF16, tag=f"wq{t}", name=f"wq{t}")
            nc.sync.dma_start(w_t[:], wqt[t * 128 : (t + 1) * 128, :])
            w_sb["q", t] = w_t
        for t in range(ND):
            x_t = xr.tile([128, QC], BF16, tag="x")
            nc.gpsimd.dma_start(x_t[:], xt[t * 128 : (t + 1) * 128, 0:QC])
            x_tiles[t] = x_t
        for t in range(ND):
            w_t = persist.tile([128, EC], BF16, tag=f"wk{t}", name=f"wk{t}")
            nc.sync.dma_start(w_t[:], wkt[t * 128 : (t + 1) * 128, :])
            w_sb["k", t] = w_t
        cos_sb = persist.tile([128, L], BF16)
        sw_sb = persist.tile([128, L], BF16)
        nc.sync.dma_start(cos_sb[:], cost[:])
        nc.sync.dma_start(sw_sb[:], sst[:])
        for t in range(ND):
            w_t = persist.tile([128, EC], BF16, tag=f"wv{t}", name=f"wv{t}")
            nc.sync.dma_start(w_t[:], wvt[t * 128 : (t + 1) * 128, :])
            w_sb["v", t] = w_t
        wo_sb = persist.tile([128, HPC * D], BF16)
        for h in range(HPC):
            nc.sync.dma_start(
                wo_sb[:, h * D : (h + 1) * D], wot[h * 128 : (h + 1) * 128, :]
            )

        # per-(batch, head, chunk) q/k bf16; per-(batch, chunk, lblock) v bf16
        qT = [[[None] * NQC for _ in range(HPC)] for _ in range(B)]
        kT = [[[None] * NQC for _ in range(HPC)] for _ in range(B)]
        v_sb = [[[None] * 4 for _ in range(NQC)] for _ in range(B)]

        def emit_x_dma(b, c, queue=None):
            for t in range(ND):
                q = queue if queue is not None else (nc.sync if t % 2 == 0 else nc.scalar)
                x_t = xr.tile([128, QC], BF16, tag="x")
                q.dma_start(
                    x_t[:],
                    xt[t * 128 : (t + 1) * 128, b * L + c * QC : b * L + (c + 1) * QC],
                )
                x_tiles[t] = x_t

        def qk_pass_mm(b, c, kind):
            raws = []
            for h in range(HPC):
                psq2 = ps("C", (128, 2 * QC), bufs=3)
                psq = psq2[:, 0:QC]
                for t in range(ND):
                    nc.tensor.matmul(
                        psq,
                        w_sb[kind, t][:, h * HD : (h + 1) * HD],
                        x_tiles[t][:],
                        start=(t == 0),
                        stop=(t == ND - 1),
                    )
                raw = st.tile([128, QC], F32, tag="raw", bufs=3)
                nc.scalar.copy(raw[:], psq)  # releases psum
                raws.append(raw)
            return raws

        def qk_rope(b, c, kind, raws):
            lsl = slice(c * QC, (c + 1) * QC)
            dst = qT if kind == "q" else kT
            for h in range(HPC):
                raw = raws[h]
                pA = st.tile([128, QC], F32, tag="pA", bufs=1)
                nc.vector.tensor_mul(pA[:], raw[:], cos_sb[:, lsl])
                tmp = st.tile([128, QC], F32, tag="tmp", bufs=2)
                nc.vector.tensor_mul(tmp[0:64, :], raw[64:128, :], sw_sb[64:128, lsl])
                nc.vector.tensor_mul(tmp[64:128, :], raw[0:64, :], sw_sb[0:64, lsl])
                dt_ = qk.tile([128, QC], BF16, tag=f"{kind}T{b}{h}_{c}")
                nc.vector.tensor_add(dt_[:], pA[:], tmp[:])
                dst[b][h][c] = dt_

        def qk_pass(b, c, kind):
            qk_rope(b, c, kind, qk_pass_mm(b, c, kind))

        def v_pass(b, c):
            psv = ps("C", (128, 2 * QC), bufs=3)
            for lb in range(4):
                for t in range(ND):
                    nc.tensor.matmul(
                        psv[:, lb * EC : (lb + 1) * EC],
                        x_tiles[t][:, lb * 128 : (lb + 1) * 128],
                        w_sb["v", t][:],
                        start=(t == 0),
                        stop=(t == ND - 1),
                    )
            for lb in range(4):
                vt = qk.tile([128, EC], BF16, tag=f"v{b}{c}_{lb}")
                nc.vector.tensor_copy(vt[:], psv[:, lb * EC : (lb + 1) * EC])
                v_sb[b][c][lb] = vt

        def attn_head(b, c, h, filler=None):
            """k-loop for one head; returns (o_raw bf16, rcp f32r [1,QC]).

            Denominators: each exp pair is summed to one [128,QC] bf16 tile on
            the Pool engine (slow but off the critical path), then folded into
            PSUM by cheap ones-matmuls on the PE, lagged two pairs so a slow
            Pool op never stalls the PE."""
            ps_o = ps("O")
            ps_u = [None]  # allocated lazily at the first denominator matmul
            prs = []

            def emit_pv(ex, p):
                for j in range(2):
                    t = 2 * p + j
                    nc.tensor.matmul(
                        ps_o[:],
                        v_sb[b][t // 4][t % 4][:, h * HD : (h + 1) * HD],
                        ex[:, j * QC : (j + 1) * QC],
                        start=(t == 0),
                        stop=(t == NKT - 1),
                    )

            def emit_dn(p):
                if ps_u[0] is None:
                    ps_u[0] = ps("U")
                nc.tensor.matmul(
                    ps_u[0][0:1, :],
                    ones_k[:],
                    prs[p][:],
                    start=(p == 0),
                    stop=(p == NKT // 2 - 1),
                )

            prev = None
            for p in range(NKT // 2):
                if filler and p >= 1:
                    filler.pop(0)()
                ps_s = ps("C", (128, 2 * QC), bufs=3)
                for j in range(2):
                    t = 2 * p + j
                    nc.tensor.matmul(
                        ps_s[:, j * QC : (j + 1) * QC],
                        kT[b][h][t // 4][:, (t % 4) * 128 : (t % 4 + 1) * 128],
                        qT[b][h][c][:],
                        start=True,
                        stop=True,
                    )
                ex = exp_pool.tile([128, 2 * QC], BF16, tag="ex")
                nc.scalar.activation(ex[:], ps_s[:], Exp, scale=SCALE)
                pr = st.tile([128, QC], BF16, tag="dps", bufs=4)
                nc.gpsimd.tensor_add(pr[:], ex[:, 0:QC], ex[:, QC : 2 * QC])
                prs.append(pr)
                if prev is not None:
                    emit_pv(prev, p - 1)
                if p >= 2:
                    emit_dn(p - 2)
                prev = ex
            emit_pv(prev, NKT // 2 - 1)
            emit_dn(NKT // 2 - 2)
            emit_dn(NKT // 2 - 1)
            while filler:
                filler.pop(0)()
            o_raw = st.tile([128, QC], BF16, tag="o_raw")
            nc.vector.tensor_copy(o_raw[:], ps_o[:])  # frees O bank
            rcp32 = st.tile([1, QC], F32, tag="rcp32", bufs=2)
            nc.vector.reciprocal_approx_fast(rcp32[:], ps_u[0][0:1, :])  # frees U
            rcp = st.tile([1, QC], F32R, tag="rcpr", bufs=2)
            nc.vector.tensor_copy(rcp[:], rcp32[:])
            return o_raw, rcp

        def norm_step(h, o_raw, rcp, onorm):
            def one():
                ps_b = ps("U")
                nc.tensor.matmul(ps_b[:], ones1[:], rcp[:], start=True, stop=True)
                nc.vector.tensor_mul(onorm[:], o_raw[:], ps_b[:])  # frees U
            return one

        def norm_head(h, o_raw, rcp):
            onorm = np_pool.tile([128, QC], BF16, tag=f"norm{h}", name=f"onorm{h}")
            norm_step(h, o_raw, rcp, onorm)()
            return onorm

        def emit_yst(b, c, e, src_ap, queue=None):
            yst = y_pool.tile([128, QC], BF16, tag="yst")
            nc.vector.tensor_copy(yst[:], src_ap)
            (queue if queue is not None else nc.sync).dma_start(
                yt[e * 128 : (e + 1) * 128, b * L + c * QC : b * L + (c + 1) * QC],
                yst[:],
            )

        def oproj_steps(b, c, norm_tiles):
            steps = []
            for p in range(ND // 2):
                def one(p=p):
                    ps_y2 = ps("C", (128, 2 * QC), bufs=3)
                    for e in (2 * p, 2 * p + 1):
                        off0 = e * 128
                        for h in range(HPC):
                            nc.tensor.matmul(
                                ps_y2[:, (e % 2) * QC : (e % 2 + 1) * QC],
                                wo_sb[:, h * D + off0 : h * D + off0 + 128],
                                norm_tiles[h][:],
                                start=(h == 0),
                                stop=(h == HPC - 1),
                            )
                    for e in (2 * p, 2 * p + 1):
                        emit_yst(b, c, e, ps_y2[:, (e % 2) * QC : (e % 2 + 1) * QC])
                steps.append(one)
            return steps

        def oproj(b, c, norm_tiles, prefill_h0=False):
            # e-pairs share one C tile; optionally emit all h0 (start)
            # matmuls of the first pairs before h1 is ready.
            pairs = [ps("C", (128, 2 * QC), bufs=3) for _ in range(2)]
            emitted = {}

            def mm(p, e, h, ps_y2):
                off = h * D + e * 128
                nc.tensor.matmul(
                    ps_y2[:, (e % 2) * QC : (e % 2 + 1) * QC],
                    wo_sb[:, off : off + 128],
                    norm_tiles[h][:],
                    start=(h == 0),
                    stop=(h == HPC - 1),
                )

            if prefill_h0:
                for p in range(2):
                    for e in (2 * p, 2 * p + 1):
                        mm(p, e, 0, pairs[p])
                        emitted[e] = True
            for p in range(ND // 2):
                ps_y2 = pairs[p] if p < 2 else ps("C", (128, 2 * QC), bufs=3)
                for e in (2 * p, 2 * p + 1):
                    if e not in emitted:
                        mm(p, e, 0, ps_y2)
                    mm(p, e, 1, ps_y2)
                for e in (2 * p, 2 * p + 1):
                    emit_yst(
                        b, c, e,
                        ps_y2[:, (e % 2) * QC : (e % 2 + 1) * QC],
                        queue=nc.sync if e % 2 == 0 else nc.gpsimd,
                    )

        # ---- schedule ----
        # phase 1: batch-0 projections
        for c in range(NQC):
            if c > 0:
                emit_x_dma(0, c, queue=nc.gpsimd)
            qk_pass(0, c, "q")
            qk_pass(0, c, "k")
            v_pass(0, c)

        # phases 2+3: previous chunk's oproj is interleaved into the next
        # chunk's k-loops, one e-pair per score-pair step. Norm broadcasts are
        # emitted only after other PE work (or deferred into the next k-loop
        # as a filler step) so their reciprocal-wait never stalls the PE.
        pend = None  # oproj steps awaiting emission
        for c in range(NQC):
            emit_x_dma(1, c)  # split sync/scalar; 36-buf ring so starts never block
            o0, r0 = attn_head(0, c, 0, filler=pend)
            qk_pass(1, c, "q")
            n0 = norm_head(0, o0, r0)
            o1, r1 = attn_head(0, c, 1)
            kraws = qk_pass_mm(1, c, "k")
            n1 = norm_head(1, o1, r1)
            pend = oproj_steps(0, c, [n0, n1])
            qk_rope(1, c, "k", kraws)
            v_pass(1, c)

        for c in range(NQC):
            if pend:
                f0 = pend[:6]
                if len(pend) == 9:  # norm1-prev at index 0 -> slot 2 for slack
                    f0 = [pend[1], pend[0]] + pend[2:6]
                rest = pend[6:]
            else:
                f0, rest = None, []
            o0, r0 = attn_head(1, c, 0, filler=f0)
            n0 = np_pool.tile([128, QC], BF16, tag="norm0", name=f"pn0_{c}")
            f1 = rest[:1] + [norm_step(0, o0, r0, n0)] + rest[1:]
            o1, r1 = attn_head(1, c, 1, filler=f1)
            n1 = np_pool.tile([128, QC], BF16, tag="norm1", name=f"pn1_{c}")
            if c < NQC - 1:
                pend = [norm_step(1, o1, r1, n1)] + oproj_steps(1, c, [n0, n1])
            else:
                ps_t = ps("C", (128, 2 * QC), bufs=3)
                for _ in range(24):  # keep HAM warm across the tail norm chain
                    nc.tensor.matmul(
                        ps_t[0:1, 0:64], ones_k[:], warm[:], start=True, stop=True
                    )
                norm_step(1, o1, r1, n1)()
                oproj(1, c, [n0, n1], prefill_h0=True)
    nc.finalize()
    return nc


def _get_nc():
    global _nc_cache
    if _nc_cache is None:
        _nc_cache = _build()
    return _nc_cache


def _prepare_in_maps(inputs):
    x = np.asarray(inputs["x"], np.float32)
    rope = np.asarray(inputs["rope_emb"], np.float32)
    wq = np.asarray(inputs["wq"], np.float32)
    wk = np.asarray(inputs["wk"], np.float32)
    wv = np.asarray(inputs["wv"], np.float32)
    wo = np.asarray(inputs["wo"], np.float32)

    import ml_dtypes

    BF = ml_dtypes.bfloat16
    xt = np.ascontiguousarray(x.reshape(B * L, D).T.astype(BF))
    cosT = np.ascontiguousarray(np.cos(rope).T)  # [HD, L]
    sinT = np.sin(rope).T  # [HD, L]
    sinsT = np.concatenate([-sinT[: HD // 2], sinT[HD // 2 :]], 0)
    # partition-rotated so rope reads stay partition-aligned on device
    ssw = np.ascontiguousarray(
        np.concatenate([sinsT[HD // 2 :], sinsT[: HD // 2]], 0)
    )
    cosT = cosT.astype(BF)
    ssw = ssw.astype(BF)

    in_maps = []
    for c in range(NCORES):
        rows = slice(c * EC, (c + 1) * EC)
        in_maps.append(
            {
                "xt": xt,
                "cost": cosT,
                "sst": ssw,
                "wqt": np.ascontiguousarray(wq[rows].T.astype(BF)),
                "wkt": np.ascontiguousarray(wk[rows].T.astype(BF)),
                "wvt": np.ascontiguousarray(wv[rows].T.astype(BF)),
                "wot": np.ascontiguousarray(wo[:, rows].T.astype(BF)),
            }
        )
    return in_maps


def kernel(**inputs):
    bo = np.asarray(inputs["bo"], np.float32)
    in_maps = _prepare_in_maps(inputs)
    nc = _get_nc()
    res = run_bass_kernel_spmd(nc, in_maps, core_ids=list(range(NCORES)))
    y_t = np.asarray(res.results[0]["yt"], dtype=np.float32)
    for c in range(1, NCORES):
        y_t += np.asarray(res.results[c]["yt"], dtype=np.float32)
    y = y_t.T.reshape(B, L, D) + bo[None, None, :]
    return y.astype(np.float32)
